# revision 33
# baseline (speedup 1.0000x reference)
"""Trainium2 Bass kernel for GroupNorm + single-head spatial self-attention
(diffusion-style attention block), data-parallel on 8 NeuronCores.

Computation (per image):
    n  = GroupNorm(x; 32 groups) * gn_scale + gn_bias          [C, N]
    q  = wq @ n + bq ; k = wk @ n + bk ; v = wv @ n + bv
    A  = softmax(q^T k / sqrt(C), axis over keys)
    out = x + wp @ (A @ v)^T + bp
Shapes: B=32, C=512, H=W=32 (N = H*W = 1024 positions); 4 images/core.

Design highlights:
  - All layouts chosen so NO transposes are needed anywhere:
    S^T = k^T q is built in [keys, queries] layout; v is built
    position-major, so AV (lhsT = v, rhs = exp(S^T)) lands channel-major
    for the output projection directly.
  - Softmax normalization is DEFERRED past both the AV matmul and the
    output projection: y = x + (wp @ AV_raw) * r + bp', applied as one
    per-element multiply at the end.
  - The denominator is a 5th "channel tile" of the AV pass: an all-ones
    DoubleRow lhsT sums exp(S^T) over keys, yielding den already
    broadcast to 128 partitions; a single lane-parallel
    reciprocal_approx_fast (18-bit) gives r.
  - Precision: GroupNorm + q/k/v projections run float32r (full-rate
    fp32 on the PE); the attention core (S^T, AV) and output projection
    run fp8e4m3 with DoubleRow (2x PE throughput). Scales are arranged
    exactly: exp carries a -ln2 bias, the projection weights are x16
    (avoids fp8 subnormals), attn-out is stored x1/4, and the ones-lhsT
    value 4.0 compensates all of it through the shared denominator.
  - Bias algebra: bk shifts every score in a softmax column equally and
    cancels exactly - never applied. bv passes through the attention
    averaging into wp@bv, folded into bp on the host. Only bq is
    applied on-device.
  - Emission is software-pipelined one image ahead; GroupNorm stats run
    on ACT/DVE/GPSIMD under the previous image's attention matmuls, and
    warm-up matmuls keep the PE HAM clock at full rate through the
    initial DMA wait.
Measured on trn2: ~220 us for the full batch, max rel err ~5.8e-3
(RMS rel ~3.9e-3) vs the fp32 reference.
"""

import numpy as np

import concourse.bacc as bacc
import concourse.tile as tile
from concourse import mybir
from concourse import bass_utils

F32 = mybir.dt.float32
F32R = mybir.dt.float32r
F8 = mybir.dt.float8e4
DR = mybir.MatmulPerfMode.DoubleRow
LN2 = 0.6931471805599453
AX = mybir.AxisListType.X
OP = mybir.AluOpType
AF = mybir.ActivationFunctionType

B, C, H, W = 32, 512, 32, 32
HW = H * W                      # 1024 spatial positions
HWH = HW // 2                   # 512 = max fp32 matmul free dim
NCORES = 8
BPC = B // NCORES               # images per core
G = 32                          # groups
GS = C // G                     # channels per group
EPS = 1e-5
P = 128
NCH = C // P                    # 4 channel chunks of 128
NPT = HW // P                   # 8 position tiles of 128
SCALE = float(C) ** -0.5


def _build():
    nc = bacc.Bacc("TRN2", target_bir_lowering=False, debug=False)

    xs = nc.dram_tensor("xs", [BPC, C, HW], F32, kind="ExternalInput")
    wqT = nc.dram_tensor("wqT", [C, C], F32R, kind="ExternalInput")
    wkT = nc.dram_tensor("wkT", [C, C], F32R, kind="ExternalInput")
    wvT = nc.dram_tensor("wvT", [C, C], F32R, kind="ExternalInput")
    wp8d = nc.dram_tensor("wp8d", [NCH // 2, P, 2, C], F8, kind="ExternalInput")
    # bias pack columns: 0=bq 1=bp' (=bp+wp@bv) 2=gn_scale 3=gn_bias
    biasp = nc.dram_tensor("biasp", [NCH, P, 4], F32, kind="ExternalInput")
    gmask = nc.dram_tensor("gmask", [NCH, P, G], F32, kind="ExternalInput")
    gmaskT = nc.dram_tensor("gmaskT", [P, C], F32, kind="ExternalInput")
    ones8md = nc.dram_tensor("ones8md", [P, 2, P], F8, kind="ExternalInput")
    ys = nc.dram_tensor("ys", [BPC, C, HW], F32, kind="ExternalOutput")

    xs_ap, ys_ap = xs.ap(), ys.ap()

    with tile.TileContext(nc) as tc:
        with (
            tc.tile_pool(name="consts", bufs=1) as cp,
            tc.tile_pool(name="work", bufs=1) as wpool,
            tc.tile_pool(name="psum", bufs=2, space="PSUM") as pp,
        ):
            st_ = {}   # mutable per-image state keyed (name, b)

            # ---- image-0 x load first so GN starts before weights land ----
            def load_x(b):
                tiles = []
                engs = (nc.sync, nc.gpsimd, nc.scalar)
                for c in range(NCH):
                    xt = wpool.tile([P, HW], F32, tag=f"x{c}", bufs=2,
                                    name=f"x_b{b}_{c}")
                    for h in range(2):
                        engs[(2 * c + h) % 3].dma_start(
                            out=xt[:, h * HWH:(h + 1) * HWH],
                            in_=xs_ap[b, c * P:(c + 1) * P,
                                      h * HWH:(h + 1) * HWH])
                    tiles.append(xt)
                st_["x", b] = tiles

            load_x(0)

            # ---- constants ----
            def const_mat(dram, tagbase):
                tiles = []
                for c in range(NCH):
                    t = cp.tile([P, C], F32R, tag=f"{tagbase}{c}",
                                name=f"{tagbase}{c}")
                    eng = nc.sync if c % 2 == 0 else nc.gpsimd
                    eng.dma_start(out=t, in_=dram.ap()[c * P:(c + 1) * P, :])
                    tiles.append(t)
                return tiles

            gm_sb = []
            for c in range(NCH):
                t = cp.tile([P, G], F32, tag=f"gm{c}", name=f"gm{c}")
                nc.sync.dma_start(out=t, in_=gmask.ap()[c])
                gm_sb.append(t)
            gmT_sb = cp.tile([P, C], F32, tag="gmT", name="gmT")
            nc.sync.dma_start(out=gmT_sb, in_=gmaskT.ap())
            bias_sb = []
            for c in range(NCH):
                t = cp.tile([P, 4], F32, tag=f"bias{c}", name=f"bias{c}")
                nc.sync.dma_start(out=t, in_=biasp.ap()[c])
                bias_sb.append(t)
            eps_sb = cp.tile([P, 1], F32, tag="eps", name="eps")
            nc.vector.memset(eps_sb, EPS)
            zero_col = cp.tile([P, 1], F32, tag="zero", name="zero")
            nc.vector.memset(zero_col, 0.0)

            wq_sb = const_mat(wqT, "wq")
            wk_sb = const_mat(wkT, "wk")
            wv_sb = const_mat(wvT, "wv")
            wp_sb = []
            for j in range(NCH // 2):
                t = cp.tile([P, 2, C], F8, tag=f"wp8{j}", name=f"wp8{j}")
                nc.sync.dma_start(out=t, in_=wp8d.ap()[j])
                wp_sb.append(t)
            ones_row = cp.tile([1, P], F32, tag="ones_row", name="ones_row")
            nc.vector.memset(ones_row, 1.0)
            ones8m = cp.tile([P, 2, P], F8, tag="ones8m", name="ones8m")
            nc.sync.dma_start(out=ones8m, in_=ones8md.ap())
            warm = pp.tile([P, HWH], F32, tag="acc1", name="warm")
            for _ in range(26):
                nc.tensor.matmul(warm[:, :P], lhsT=ones_row[:1, :],
                                 rhs=ones_row[:1, :], start=True, stop=True)
            lnh_col = cp.tile([P, 1], F32, tag="lnh", name="lnh")
            nc.vector.memset(lnh_col, -LN2)

            # ---- per-image phases ----
            def gn_stats(b):
                x_sb = st_["x", b]
                stt = []
                for c in range(NCH):
                    s = wpool.tile([P, 2], F32, tag=f"st{c}", name=f"st_b{b}_{c}")
                    nc.vector.reduce_sum(out=s[:, 0:1], in_=x_sb[c], axis=AX)
                    scr = wpool.tile([P, HW], F32, tag="sqscr", bufs=2,
                                     name=f"sqscr_b{b}_{c}")
                    nc.scalar.activation(out=scr, in_=x_sb[c], func=AF.Square,
                                         bias=zero_col, accum_out=s[:, 1:2])
                    stt.append(s)

                gp = pp.tile([G, 2], F32, tag="acc1", name=f"gp_b{b}")
                for c in range(NCH):
                    nc.tensor.matmul(gp, lhsT=gm_sb[c], rhs=stt[c],
                                     start=(c == 0), stop=(c == NCH - 1))

                # gmr: col0 = group mean, col1 = group rstd (rows >= G zero)
                gmr = wpool.tile([P, 2], F32, tag="gmr", name=f"gmr_b{b}")
                nc.vector.memset(gmr, 0.0)
                nc.vector.tensor_scalar(gmr[:G, 0:1], gp[:G, 0:1],
                                        1.0 / (GS * HW), None, OP.mult)
                e2 = wpool.tile([P, 1], F32, tag="e2", name=f"e2_b{b}")
                nc.vector.tensor_scalar(e2[:G], gp[:G, 1:2],
                                        1.0 / (GS * HW), None, OP.mult)
                m2 = wpool.tile([P, 1], F32, tag="m2", name=f"m2_b{b}")
                nc.vector.tensor_mul(m2[:G], gmr[:G, 0:1], gmr[:G, 0:1])
                var = wpool.tile([P, 1], F32, tag="var", name=f"var_b{b}")
                nc.vector.tensor_sub(var[:G], e2[:G], m2[:G])
                sd = wpool.tile([P, 1], F32, tag="sd", name=f"sd_b{b}")
                nc.scalar.activation(out=sd[:G], in_=var[:G], func=AF.Sqrt,
                                     bias=eps_sb[:G])
                nc.vector.reciprocal(out=gmr[:G, 1:2], in_=sd[:G])
                st_["gmr", b] = gmr

            def normalize(b):
                x_sb, gmr = st_["x", b], st_.pop(("gmr", b))
                n_sb = []
                for c in range(NCH):
                    bc = pp.tile([P, 2], F32, tag="acc1", name=f"bc_b{b}_{c}")
                    nc.tensor.matmul(bc, lhsT=gmT_sb[:, c * P:(c + 1) * P],
                                     rhs=gmr, start=True, stop=True)
                    a = wpool.tile([P, 1], F32, tag=f"a{c}", name=f"a_b{b}_{c}")
                    nc.vector.tensor_mul(a, bc[:, 1:2], bias_sb[c][:, 2:3])
                    gt = wpool.tile([P, 1], F32, tag=f"gt{c}", name=f"gt_b{b}_{c}")
                    nc.vector.tensor_mul(gt, bc[:, 0:1], a)
                    bb = wpool.tile([P, 1], F32, tag=f"bb{c}", name=f"bb_b{b}_{c}")
                    nc.vector.tensor_sub(bb, bias_sb[c][:, 3:4], gt)
                    nt = wpool.tile([P, HW], F32R, tag=f"n{c}", name=f"n_b{b}_{c}")
                    neng = nc.vector if b == 0 else nc.gpsimd
                    neng.tensor_scalar(nt, x_sb[c], a, bb, OP.mult, OP.add)
                    n_sb.append(nt)
                st_["n", b] = n_sb

            def qkv(b):
                n_sb = st_.pop(("n", b))
                # q/k evacuate into fp8 DoubleRow pair tiles [P, 2, HW]:
                # logical contraction row (2j+i)*128+p lives at [p, i, :] of
                # pair j. q gets +bq (DVE); k's bias cancels in softmax (ACT).
                for (w_t, tagbase) in ((wq_sb, "q"), (wk_sb, "k")):
                    dst = []
                    for j in range(NCH // 2):
                        t8t = wpool.tile([P, 2, HW], F8, tag=f"{tagbase}8{j}",
                                         name=f"{tagbase}8_b{b}_{j}")
                        dst.append(t8t)
                    for o in range(NCH):
                        acc = pp.tile([P, HW], F32, tag="acc2", bufs=3,
                                      name=f"{tagbase}acc_b{b}_{o}")
                        for c in range(NCH):
                            for h in range(2):
                                nc.tensor.matmul(
                                    acc[:, h * HWH:(h + 1) * HWH],
                                    lhsT=w_t[c][:, o * P:(o + 1) * P],
                                    rhs=n_sb[c][:, h * HWH:(h + 1) * HWH],
                                    start=(c == 0), stop=(c == NCH - 1))
                        out8 = dst[o // 2][:, o % 2, :]
                        if tagbase == "q":
                            nc.vector.tensor_scalar(out8, acc,
                                                    bias_sb[o][:, 0:1],
                                                    None, OP.add)
                        else:
                            nc.vector.tensor_copy(out=out8, in_=acc)
                    st_[tagbase, b] = dst
                # v-projection groups interleaved with the S^T groups so the
                # exp chain (8 x ~1.1us serial on ACT) starts ~7us earlier and
                # finishes before AV needs it.
                v_sb = []
                for j in range(NPT // 2):
                    v_sb.append(wpool.tile([P, 2, HWH], F8, tag=f"v8{j}",
                                           name=f"v8_b{b}_{j}"))
                e_sb = []
                for j in range(NPT // 2):
                    e_sb.append(wpool.tile([P, 2, HW], F8, tag=f"e8{j}",
                                           name=f"e8_b{b}_{j}"))
                q8_sb, k8_sb = st_.pop(("q", b)), st_.pop(("k", b))
                for t8 in range(NPT):
                    acc = pp.tile([P, HWH], F32, tag="acc1", name=f"vacc_b{b}_{t8}")
                    for c in range(NCH):
                        nc.tensor.matmul(acc,
                                         lhsT=n_sb[c][:, t8 * P:(t8 + 1) * P],
                                         rhs=wv_sb[c],
                                         start=(c == 0), stop=(c == NCH - 1))
                    nc.scalar.copy(v_sb[t8 // 2][:, t8 % 2, :], acc)

                    m = t8
                    sacc = pp.tile([P, HW], F32, tag="acc2", bufs=3, name=f"sacc_b{b}_{m}")
                    for c in range(NCH // 2):
                        for h in range(2):
                            nc.tensor.matmul(
                                sacc[:, h * HWH:(h + 1) * HWH],
                                lhsT=k8_sb[c][:, :, m * P:(m + 1) * P],
                                rhs=q8_sb[c][:, :, h * HWH:(h + 1) * HWH],
                                start=(c == 0), stop=(c == NCH // 2 - 1),
                                perf_mode=DR)
                    # exp scaled by 1/2 (bias -ln2) for fp8e4 range headroom;
                    # cancels exactly against the denominator. Half-granular
                    # so AV DoubleRow mms and PSUM banks unblock sooner.
                    for eh in range(2):
                        esl = slice(eh * HWH, (eh + 1) * HWH)
                        nc.scalar.activation(out=e_sb[m // 2][:, m % 2, esl],
                                             in_=sacc[:, esl], func=AF.Exp,
                                             bias=lnh_col, scale=SCALE)
                st_["v", b] = v_sb
                st_["e", b] = e_sb

            def den_phase(b):
                pass

            def av_den(b):
                e_sb, v_sb = st_["e", b], st_.pop(("v", b))
                o_sb = []
                for ct in range(NCH):
                    acc = pp.tile([P, HW], F32, tag="acc2", bufs=3, name=f"oacc_b{b}_{ct}")
                    for m in range(NPT // 2):
                        for h in range(2):
                            nc.tensor.matmul(
                                acc[:, h * HWH:(h + 1) * HWH],
                                lhsT=v_sb[m][:, :, ct * P:(ct + 1) * P],
                                rhs=e_sb[m][:, :, h * HWH:(h + 1) * HWH],
                                start=(m == 0), stop=(m == NPT // 2 - 1),
                                perf_mode=DR)
                    j, i = divmod(ct, 2)
                    if i == 0:
                        o_sb.append(wpool.tile([P, 2, HW], F8, tag=f"o8{j}",
                                               name=f"o8_b{b}_{j}"))
                    # scale 1/4 keeps |attn-raw| inside e4m3 range; exactly
                    # compensated by the 4.0-valued denominator lhsT.
                    nc.scalar.activation(out=o_sb[j][:, i, :], in_=acc,
                                         func=AF.Copy, scale=0.25)
                st_["o", b] = o_sb
                # 5th "channel tile": all-ones lhsT sums E over keys, giving
                # the softmax denominator already broadcast to 128 partitions.
                dbc = pp.tile([P, HW], F32, tag="acc2", bufs=3, name=f"dbc_b{b}")
                for m in range(NPT // 2):
                    for h in range(2):
                        nc.tensor.matmul(
                            dbc[:, h * HWH:(h + 1) * HWH],
                            lhsT=ones8m[:, :, :],
                            rhs=e_sb[m][:, :, h * HWH:(h + 1) * HWH],
                            start=(m == 0), stop=(m == NPT // 2 - 1),
                            perf_mode=DR)
                st_.pop(("e", b))
                r_sb = wpool.tile([P, HW], F32, tag="r", name=f"r_b{b}")
                nc.vector.reciprocal_approx_fast(out=r_sb, in_=dbc)
                st_["r", b] = r_sb

            def rbc(b):
                pass

            def proj(b):
                o_sb = st_.pop(("o", b))
                x_sb = st_.pop(("x", b))
                for o in range(NCH):
                    acc = pp.tile([P, HW], F32, tag="acc2", bufs=3, name=f"pacc_b{b}_{o}")
                    for c in range(NCH // 2):
                        for h in range(2):
                            nc.tensor.matmul(
                                acc[:, h * HWH:(h + 1) * HWH],
                                lhsT=wp_sb[c][:, :, o * P:(o + 1) * P],
                                rhs=o_sb[c][:, :, h * HWH:(h + 1) * HWH],
                                start=(c == 0), stop=(c == NCH // 2 - 1),
                                perf_mode=DR)
                    if o == 0:
                        r_sb = st_.pop(("r", b))
                    t1 = wpool.tile([P, HW], F32, tag="t1", bufs=2,
                                    name=f"t1_b{b}_{o}")
                    yt = wpool.tile([P, HW], F32, tag=f"y{o}", name=f"y_b{b}_{o}")
                    for h in range(2):
                        sl = slice(h * HWH, (h + 1) * HWH)
                        nc.vector.tensor_mul(t1[:, sl], acc[:, sl], r_sb[:, sl])
                        nc.vector.scalar_tensor_tensor(
                            out=yt[:, sl], in0=t1[:, sl],
                            scalar=bias_sb[o][:, 1:2], in1=x_sb[o][:, sl],
                            op0=OP.add, op1=OP.add)
                        oeng = nc.sync if (o + h) % 2 == 0 else nc.gpsimd
                        oeng.dma_start(out=ys_ap[b, o * P:(o + 1) * P, sl],
                                       in_=yt[:, sl])

            # ---- software-pipelined emission, one image ahead ----
            gn_stats(0)
            normalize(0)
            qkv(0)
            for b in range(BPC):
                if b + 1 < BPC:
                    load_x(b + 1)
                av_den(b)
                den_phase(b)
                if b + 1 < BPC:
                    gn_stats(b + 1)
                    normalize(b + 1)
                proj(b)
                if b + 1 < BPC:
                    qkv(b + 1)

    nc.compile()
    return nc


_NC = None


def _get_nc():
    global _NC
    if _NC is None:
        _NC = _build()
    return _NC


def _host_inputs(x, gn_scale, gn_bias, wq, bq, wk, bk, wv, bv, wp, bp):
    x = np.ascontiguousarray(np.asarray(x, np.float32).reshape(B, C, HW))
    f = lambda t: np.ascontiguousarray(np.asarray(t, np.float32))
    gn_scale, gn_bias = f(gn_scale), f(gn_bias)
    bq, bv, bp = f(bq), f(bv), f(bp)
    wq, wk, wv, wp = f(wq), f(wk), f(wv), f(wp)

    bp_eff = bp + wp @ bv  # v-bias passes through softmax-averaging intact
    biasp = np.stack([bq, bp_eff, gn_scale, gn_bias], 1).reshape(NCH, P, 4)
    ch = np.arange(C)
    gmask_full = (ch[:, None] // GS == np.arange(G)[None, :]).astype(np.float32)
    gmask = np.ascontiguousarray(gmask_full.reshape(NCH, P, G))
    gmaskT = np.zeros((P, C), np.float32)
    gmaskT[:G, :] = gmask_full.T
    def dr_pack(w):
        wt = (w.T * 16.0).astype(mybir.dt.np(F8))
        wt = wt.reshape(NCH // 2, 2, P, C).transpose(0, 2, 1, 3)
        return np.ascontiguousarray(wt)

    common = {
        "wqT": np.ascontiguousarray(wq.T),
        "wkT": np.ascontiguousarray(wk.T),
        "wvT": np.ascontiguousarray(wv.T),
        "wp8d": dr_pack(wp),
        "biasp": np.ascontiguousarray(biasp),
        "gmask": gmask,
        "gmaskT": gmaskT,
        "ones8md": np.full((P, 2, P), 4.0, mybir.dt.np(F8)),
    }
    in_maps = []
    for i in range(NCORES):
        m = dict(common)
        m["xs"] = np.ascontiguousarray(x[i * BPC:(i + 1) * BPC])
        in_maps.append(m)
    return in_maps


def _run(in_maps, trace=False):
    nc = _get_nc()
    return bass_utils.run_bass_kernel_spmd(nc, in_maps, list(range(NCORES)),
                                           trace=trace)


def kernel(**inputs):
    in_maps = _host_inputs(**inputs)
    try:
        res = _run(in_maps, trace=False)
    except Exception:
        # transient device faults (e.g. NRT_EXEC_UNIT_UNRECOVERABLE) clear
        # on re-execution; one retry costs nothing when the first run works
        res = _run(in_maps, trace=False)
    y = np.concatenate([r["ys"] for r in res.results], axis=0)
    return y.reshape(B, C, H, W)


def run_traced(**inputs):
    """Like kernel() but with NTFF tracing; returns (y, exec_time_ns)."""
    in_maps = _host_inputs(**inputs)
    res = _run(in_maps, trace=True)
    y = np.concatenate([r["ys"] for r in res.results], axis=0)
    return y.reshape(B, C, H, W), res.exec_time_ns
